# revision 2
# baseline (speedup 1.0000x reference)
"""Mixtral MoE layer (T=1024, H=1024, I=2048, E=8, top-2) on 8 Trainium2 cores.

Strategy: expert-parallel. Core c owns expert c's full FFN (w1/w3/w2).
The router (softmax + top-2 + renormalize -> combine[T, E]) runs on host;
core c computes outT_c = w2_c @ (silu(w1_c @ x^T) * (w3_c @ x^T)) scaled by
combine[:, c] along tokens, then an on-device ReduceScatter sums the eight
[H, T] partials and leaves shard c (rows c*128:(c+1)*128) on core c. Host
concatenates the shards and transposes back to [T, H].

Matmuls run as float32r (TF32-like, full PE rate at N=512). All weight
tiles are pre-transposed on host so every DMA is contiguous per partition.
"""

import os
import sys

sys.path.insert(0, "/opt/trn_rl_repo")

import numpy as np

import concourse.bacc as bacc
import concourse.tile as tile
from concourse import mybir
from concourse.bass_utils import run_bass_kernel_spmd

F32 = mybir.dt.float32
F32R = mybir.dt.float32r

T = 1024   # tokens
H = 1024   # hidden
I = 2048   # intermediate
E = 8      # experts
TOPK = 2
P = 128
NKH = H // P     # 8  h-tiles (up-proj contraction)
NI = I // P      # 16 i-tiles
NH = H // P      # 8  h-tiles (down-proj output)
NT = T // 512    # 2  moving-operand blocks
N_CORES = 8

_NC_CACHE = {}


def build_nc(collective: bool = True):
    key = ("moe", collective)
    if key in _NC_CACHE:
        return _NC_CACHE[key]

    nc = bacc.Bacc(None, target_bir_lowering=False, num_devices=N_CORES)

    xt_in = nc.declare_dram_parameter("xt", [H, T], F32, isOutput=False)
    comb_in = nc.declare_dram_parameter("comb", [P, T], F32, isOutput=False)
    w1q_in = nc.declare_dram_parameter("w1q", [NI, P, NKH * P], F32, isOutput=False)
    w3q_in = nc.declare_dram_parameter("w3q", [NI, P, NKH * P], F32, isOutput=False)
    w2q_in = nc.declare_dram_parameter("w2q", [NI, P, H], F32, isOutput=False)
    if collective:
        outp = nc.declare_dram_parameter("outp", [P, T], F32, isOutput=True)
    else:
        outp = nc.declare_dram_parameter("outp", [H, T], F32, isOutput=True)

    with tile.TileContext(nc) as tc:
        with (
            tc.tile_pool(name="persist", bufs=1) as persist,
            tc.tile_pool(name="stream", bufs=2) as stream,
            tc.tile_pool(name="psum", bufs=1, space="PSUM") as psum,
            tc.tile_pool(name="dram", bufs=1, space="DRAM") as dram,
        ):
            # ---- stage 0: resident activations ----
            xt_sb = []
            for kh in range(NKH):
                t_ = persist.tile([P, T], F32R, name=f"xt_{kh}", tag=f"xt_{kh}")
                nc.sync.dma_start(out=t_[:], in_=xt_in[kh * P : (kh + 1) * P, :].bitcast(F32R))
                xt_sb.append(t_)
            comb_sb = persist.tile([P, T], F32, name="comb_sb", tag="comb_sb")
            nc.sync.dma_start(out=comb_sb[:], in_=comb_in[:, :])

            w2_sb = []
            act_sb = []

            # ---- stage 1: up-projections + SwiGLU, per i-tile ----
            for it in range(NI):
                w1sb = stream.tile([P, NKH * P], F32R, name=f"w1sb_{it}", tag="w1sb", bufs=2)
                w3sb = stream.tile([P, NKH * P], F32R, name=f"w3sb_{it}", tag="w3sb", bufs=2)
                w2sb = persist.tile([P, H], F32R, name=f"w2sb_{it}", tag=f"w2sb_{it}")
                for half in range(2):
                    s = slice(half * (NKH * P // 2), (half + 1) * (NKH * P // 2))
                    nc.sync.dma_start(out=w1sb[:, s], in_=w1q_in[it][:, s].bitcast(F32R))
                    nc.sync.dma_start(out=w3sb[:, s], in_=w3q_in[it][:, s].bitcast(F32R))
                    nc.sync.dma_start(out=w2sb[:, s], in_=w2q_in[it][:, s].bitcast(F32R))
                w2_sb.append(w2sb)

                ph1 = psum.tile([P, T], F32, name=f"ph1_{it}", tag="ph1", bufs=1)
                ph3 = psum.tile([P, T], F32, name=f"ph3_{it}", tag="ph3", bufs=1)
                for ph, wsb in ((ph1, w1sb), (ph3, w3sb)):
                    for kh in range(NKH):
                        lhsT = wsb[:, kh * P : (kh + 1) * P]
                        for tb in range(NT):
                            ts = slice(tb * 512, (tb + 1) * 512)
                            nc.tensor.matmul(
                                ph[:, ts], lhsT, xt_sb[kh][:, ts],
                                start=(kh == 0), stop=(kh == NKH - 1),
                            )

                silu1 = stream.tile([P, T], F32, name=f"silu_{it}", tag="silu", bufs=2)
                nc.scalar.activation(silu1[:], ph1[:], mybir.ActivationFunctionType.Silu)
                act = persist.tile([P, T], F32R, name=f"act_{it}", tag=f"act_{it}")
                nc.vector.tensor_mul(act[:], silu1[:], ph3[:])
                act_sb.append(act)

            # ---- stage 2: down-projection, per h-tile; fold in combine scale ----
            if collective:
                ar_in = dram.tile([H, T], F32, name="ar_in")
            out_dst = ar_in if collective else outp
            for ht in range(NH):
                po = psum.tile([P, T], F32, name=f"po_{ht}", tag="po", bufs=2)
                for ii in range(NI):
                    lhsT = w2_sb[ii][:, ht * P : (ht + 1) * P]
                    for tb in range(NT):
                        ts = slice(tb * 512, (tb + 1) * 512)
                        nc.tensor.matmul(
                            po[:, ts], lhsT, act_sb[ii][:, ts],
                            start=(ii == 0), stop=(ii == NI - 1),
                        )
                outsb = stream.tile([P, T], F32, name=f"outsb_{ht}", tag="outsb", bufs=2)
                nc.vector.tensor_mul(outsb[:], po[:], comb_sb[:])
                nc.sync.dma_start(out=out_dst[ht * P : (ht + 1) * P, :], in_=outsb[:])

            # ---- stage 3: cross-core reduction ----
            if collective:
                rs_out = dram.tile([P, T], F32, name="rs_out")
                nc.gpsimd.collective_compute(
                    "ReduceScatter",
                    mybir.AluOpType.add,
                    replica_groups=[list(range(N_CORES))],
                    ins=[ar_in.opt()],
                    outs=[rs_out.opt()],
                )
                nc.gpsimd.dma_start(out=outp[:, :], in_=rs_out[:])

    nc.compile()
    _NC_CACHE[key] = nc
    return nc


def _route(x: np.ndarray, gw: np.ndarray) -> np.ndarray:
    """Host router: softmax over expert logits, top-2, renormalize.

    Returns combine [T, E] f32 with zeros for unselected experts.
    """
    logits = x @ gw.T                                   # [T, E]
    logits = logits - logits.max(axis=1, keepdims=True)
    ex = np.exp(logits)
    rw = ex / ex.sum(axis=1, keepdims=True)
    idx = np.argsort(-rw, axis=1, kind="stable")[:, :TOPK]
    v = np.take_along_axis(rw, idx, axis=1)
    v = v / v.sum(axis=1, keepdims=True)
    combine = np.zeros((T, E), np.float32)
    np.put_along_axis(combine, idx, v.astype(np.float32), axis=1)
    return combine


def prepare_in_maps(index, hidden_states, gate_w, ws):
    x = np.ascontiguousarray(np.asarray(hidden_states, dtype=np.float32))
    li = int(index)
    gw = np.asarray(gate_w, dtype=np.float32)[li]       # [E, H]
    wsl = np.asarray(ws, dtype=np.float32)[li]          # [E, 3*I*H]

    combine = _route(x, gw)
    xt = np.ascontiguousarray(x.T)                      # [H, T]

    in_maps = []
    for c in range(N_CORES):
        w1 = wsl[c, : I * H].reshape(I, H)
        w3 = wsl[c, I * H : 2 * I * H].reshape(I, H)
        w2 = wsl[c, 2 * I * H :].reshape(H, I)
        # w1q[it, p, kh*P+m] = w1[it*P+m, kh*P+p]  (lhsT tiles, contiguous per partition)
        w1q = np.ascontiguousarray(
            w1.reshape(NI, P, NKH, P).transpose(0, 3, 2, 1)
        ).reshape(NI, P, NKH * P)
        w3q = np.ascontiguousarray(
            w3.reshape(NI, P, NKH, P).transpose(0, 3, 2, 1)
        ).reshape(NI, P, NKH * P)
        # w2q[it, p, h] = w2[h, it*P+p]
        w2q = np.ascontiguousarray(w2.T).reshape(NI, P, H)
        comb_c = np.ascontiguousarray(
            np.broadcast_to(combine[:, c][None, :], (P, T))
        )
        in_maps.append({"xt": xt, "comb": comb_c, "w1q": w1q, "w3q": w3q, "w2q": w2q})
    return in_maps


def run_device(in_maps, collective=True, **spmd_kwargs):
    nc = build_nc(collective)
    return run_bass_kernel_spmd(nc, in_maps, list(range(N_CORES)), **spmd_kwargs)


def assemble_output(results, collective=True) -> np.ndarray:
    if collective:
        outT = np.concatenate([results[c]["outp"] for c in range(N_CORES)], axis=0)
    else:
        outT = np.zeros((H, T), np.float64)
        for c in range(N_CORES):
            outT += results[c]["outp"].astype(np.float64)
    return np.ascontiguousarray(outT.T).astype(np.float32)


def kernel(index, hidden_states, experts_cache, gate_w, ws) -> np.ndarray:
    collective = os.environ.get("BASS_MOE_NO_CC", "") != "1"
    in_maps = prepare_in_maps(index, hidden_states, gate_w, ws)
    res = run_device(in_maps, collective=collective)
    return assemble_output(res.results, collective=collective)
